# revision 1
# baseline (speedup 1.0000x reference)
"""ColorQuantizer (VQ nearest-palette-color) Trainium2 Bass kernel.

Reference semantics: out[b,:,h,w] = palette[argmin_k ||(x+0.01*noise)[b,:,h,w] - palette[k]||]
(The straight-through estimator is numerically the identity on the forward pass.)

Sharding: pure data parallel over batch (32 -> 8 cores x 4), palette replicated.
"""
import sys

sys.path.insert(0, "/opt/trn_rl_repo")

import numpy as np

import concourse.bacc as bacc
import concourse.mybir as mybir
from concourse.tile import TileContext
from concourse.bass_utils import run_bass_kernel_spmd

# Problem constants (hardcoded per harness contract)
B, C, H, W = 32, 3, 512, 512
K = 16
N_CORES = 8
B_PER_CORE = B // N_CORES  # 4
NOISE_SCALE = 0.01

F = 1024          # free-dim elements per tile
HROWS = 256       # h-rows consumed per tile (128 partitions x 2 rows)
T_PER_PLANE = H // HROWS  # 2 tiles per (batch, h) plane split

_DT = mybir.dt.float32


def _plane_ap(t_dram, b, c, t):
    """[128, F] view of channel plane c of batch b, h-rows [t*256,(t+1)*256)."""
    return t_dram[b, c, t * HROWS : (t + 1) * HROWS, :].rearrange(
        "(p a) w -> p (a w)", p=128
    )


def _build(repeat=1):
    nc = bacc.Bacc("TRN2", target_bir_lowering=False, debug=False,
                   num_devices=N_CORES)
    x = nc.dram_tensor("x", [B_PER_CORE, C, H, W], _DT, kind="ExternalInput").ap()
    n = nc.dram_tensor("noise", [B_PER_CORE, C, H, W], _DT, kind="ExternalInput").ap()
    pal = nc.dram_tensor("palette", [K, C], _DT, kind="ExternalInput").ap()
    o = nc.dram_tensor("out", [B_PER_CORE, C, H, W], _DT, kind="ExternalOutput").ap()

    Alu = mybir.AluOpType
    Act = mybir.ActivationFunctionType

    with TileContext(nc) as tc:
        with (
            tc.tile_pool(name="const", bufs=1) as cpool,
            tc.tile_pool(name="io", bufs=3) as io,
            tc.tile_pool(name="scratch", bufs=3) as sc,
            tc.tile_pool(name="carry", bufs=2) as carry,
        ):
            # palette -> SBUF [128, 48] broadcast across partitions; col = k*3+c
            pal_sb = cpool.tile([128, K * C], _DT)
            nc.sync.dma_start(
                out=pal_sb[:],
                in_=pal.rearrange("(o k) c -> o (k c)", o=1).to_broadcast([128, K * C]),
            )
            # negated palette for ACT Square bias
            npal_sb = cpool.tile([128, K * C], _DT)
            nc.vector.tensor_scalar(
                out=npal_sb[:], in0=pal_sb[:], scalar1=-1.0, scalar2=None,
                op0=Alu.mult)

            for rep in range(repeat):
              for b in range(B_PER_CORE):
                for t in range(T_PER_PLANE):
                    xt = [io.tile([128, F], _DT, tag=f"x{c}", name=f"xt{c}") for c in range(C)]
                    nt = [io.tile([128, F], _DT, tag=f"n{c}", name=f"nt{c}") for c in range(C)]
                    for c in range(C):
                        nc.sync.dma_start(out=xt[c][:], in_=_plane_ap(x, b, c, t))
                        nc.sync.dma_start(out=nt[c][:], in_=_plane_ap(n, b, c, t))

                    # y_c = x_c + NOISE_SCALE * n_c
                    yt = [sc.tile([128, F], _DT, tag=f"y{c}", name=f"yt{c}") for c in range(C)]
                    for c in range(C):
                        nc.vector.scalar_tensor_tensor(
                            out=yt[c][:], in0=nt[c][:], scalar=NOISE_SCALE,
                            in1=xt[c][:], op0=Alu.mult, op1=Alu.add)

                    m = carry.tile([128, F], _DT, tag="m")
                    mask = carry.tile([128, F], mybir.dt.uint8, tag="mask")
                    ot = [carry.tile([128, F], _DT, tag=f"o{c}", name=f"ot{c}") for c in range(C)]

                    for k in range(K):
                        q = [sc.tile([128, F], _DT, tag=f"q{c}", name=f"qt{c}") for c in range(C)]
                        for c in range(C):
                            # q_c = (y_c - p_kc)^2
                            nc.scalar.activation(
                                out=q[c][:], in_=yt[c][:], func=Act.Square,
                                bias=npal_sb[:, k * C + c : k * C + c + 1],
                                scale=1.0)
                        if k == 0:
                            # d -> m directly; out_c = palette color 0
                            nc.vector.tensor_tensor(
                                out=m[:], in0=q[0][:], in1=q[1][:], op=Alu.add)
                            nc.vector.tensor_tensor(
                                out=m[:], in0=m[:], in1=q[2][:], op=Alu.add)
                            for c in range(C):
                                nc.vector.tensor_copy(
                                    out=ot[c][:],
                                    in_=pal_sb[:, c : c + 1].to_broadcast([128, F]))
                        else:
                            d = sc.tile([128, F], _DT, tag="d")
                            nc.vector.tensor_tensor(
                                out=d[:], in0=q[0][:], in1=q[1][:], op=Alu.add)
                            nc.vector.tensor_tensor(
                                out=d[:], in0=d[:], in1=q[2][:], op=Alu.add)
                            # strict less => first-wins tie-breaking
                            nc.vector.tensor_tensor(
                                out=mask[:], in0=d[:], in1=m[:], op=Alu.is_lt)
                            nc.vector.tensor_tensor(
                                out=m[:], in0=m[:], in1=d[:], op=Alu.min)
                            for c in range(C):
                                nc.vector.copy_predicated(
                                    out=ot[c][:], mask=mask[:],
                                    data=pal_sb[:, k * C + c : k * C + c + 1]
                                    .to_broadcast([128, F]))

                    for c in range(C):
                        nc.sync.dma_start(out=_plane_ap(o, b, c, t), in_=ot[c][:])

    nc.compile()
    return nc


_NC_CACHE = {}


def _get_nc(repeat=1):
    if repeat not in _NC_CACHE:
        _NC_CACHE[repeat] = _build(repeat)
    return _NC_CACHE[repeat]


def kernel(x, noise, palette):
    x = np.ascontiguousarray(np.asarray(x, dtype=np.float32))
    noise = np.ascontiguousarray(np.asarray(noise, dtype=np.float32))
    palette = np.ascontiguousarray(np.asarray(palette, dtype=np.float32))
    nc = _get_nc()
    in_maps = [
        {
            "x": x[i * B_PER_CORE : (i + 1) * B_PER_CORE],
            "noise": noise[i * B_PER_CORE : (i + 1) * B_PER_CORE],
            "palette": palette,
        }
        for i in range(N_CORES)
    ]
    res = run_bass_kernel_spmd(nc, in_maps, list(range(N_CORES)))
    out = np.concatenate([res.results[i]["out"] for i in range(N_CORES)], axis=0)
    return out.astype(np.float32, copy=False)


if __name__ == "__main__":
    rng = np.random.default_rng(0)
    x = rng.random((B, C, H, W), dtype=np.float32)
    noise = rng.standard_normal((B, C, H, W), dtype=np.float32)
    palette = rng.random((K, C), dtype=np.float32)
    out = kernel(x, noise, palette)
    y = np.transpose(x + NOISE_SCALE * noise, (0, 2, 3, 1)).reshape(-1, 3)
    d = ((y[:, None, :] - palette[None, :, :]) ** 2).sum(-1)
    idx = np.argmin(d, axis=-1)
    expect = np.transpose(
        palette[idx].reshape(B, H, W, C), (0, 3, 1, 2))
    err = np.abs(out - expect).max()
    print("abs max err vs numpy argmin:", err)
    mism = (out != expect).any(axis=1).sum()
    print("mismatched pixels:", mism, "/", B * H * W)



# revision 2
# speedup vs baseline: 30.5074x; 30.5074x over previous
"""ColorQuantizer v3: per-instruction-overhead-optimized.

Empirical backend model: each instruction costs ~35-90us dispatch + ~6ns/free-el;
engines (DVE/ACT/DMA) run in parallel. So: minimize per-engine instruction
count, fd=4096 tiles (2 batches x 2048), optional For_i loop over palette
entries, 3MB DMAs, interleaved copy_predicated for selection.

Tile layouts per half (2 batches):
  xt/nt [128, 12288]: col = b*6144 + c*2048 + f   (planar per (b,c))
  ot    [128, 12288]: col = (b*2048+f)*3 + c      (pixel-interleaved)
"""
import sys

sys.path.insert(0, "/opt/trn_rl_repo")

import numpy as np

import concourse.bacc as bacc
import concourse.mybir as mybir
from concourse.tile import TileContext
from concourse.bass import ds
from concourse.bass_utils import run_bass_kernel_spmd

B, C, H, W = 32, 3, 512, 512
K = 16
N_CORES = 8
B_PER_CORE = B // N_CORES  # 4
NOISE_SCALE = 0.01

FD = 4096               # pixels per partition per half (2 batches x 2048)
B_PER_UNIT = 2
UNITS = B_PER_CORE // B_PER_UNIT  # 2
FB = 2048               # pixels per partition per batch

_DT = mybir.dt.float32
Alu = mybir.AluOpType
Act = mybir.ActivationFunctionType


def _batch_ap(t, b):
    """DRAM view [128, c, (a w)] of batch b: partition p <- h-rows [4p,4p+4)."""
    return t[b].rearrange("c (p a) w -> p c (a w)", p=128)


def _build(repeat=1, loop=False, kloop=False):
    nc = bacc.Bacc("TRN2", target_bir_lowering=False, debug=False,
                   num_devices=N_CORES)
    x = nc.dram_tensor("x", [B_PER_CORE, C, H, W], _DT, kind="ExternalInput").ap()
    n = nc.dram_tensor("noise", [B_PER_CORE, C, H, W], _DT, kind="ExternalInput").ap()
    pal = nc.dram_tensor("palette", [K, C], _DT, kind="ExternalInput").ap()
    o = nc.dram_tensor("out", [B_PER_CORE, C, H, W], _DT, kind="ExternalOutput").ap()

    with TileContext(nc) as tc:
        with (
            tc.tile_pool(name="const", bufs=1) as cpool,
            tc.tile_pool(name="io", bufs=1) as io,
            tc.tile_pool(name="work", bufs=1) as wk,
        ):
            # palette -> SBUF [128, 48] broadcast across partitions; col = k*3+c
            pal_sb = cpool.tile([128, K * C], _DT)
            nc.sync.dma_start(
                out=pal_sb[:],
                in_=pal.rearrange("(o k) c -> o (k c)", o=1).to_broadcast([128, K * C]),
            )
            npal_sb = cpool.tile([128, K * C], _DT)
            nc.vector.tensor_scalar(
                out=npal_sb[:], in0=pal_sb[:], scalar1=-1.0, scalar2=None,
                op0=Alu.mult)

            def qview(t, c):
                """channel-c view [128, b=2, 2048] of a (b,c,f)-layout tile."""
                return t[:].rearrange(
                    "p (b c f) -> p c b f", b=B_PER_UNIT, c=C)[:, c]

            def body(rep):
                for u in range(UNITS):
                    xt = io.tile([128, C * FD], _DT, tag="x", name="xt")
                    nt = io.tile([128, C * FD], _DT, tag="n", name="nt")
                    for b in range(B_PER_UNIT):
                        gb = B_PER_UNIT * u + b
                        sl = slice(b * C * FB, (b + 1) * C * FB)
                        nc.sync.dma_start(
                            out=xt[:, sl].rearrange("p (c f) -> p c f", c=C),
                            in_=_batch_ap(x, gb))
                        nc.sync.dma_start(
                            out=nt[:, sl].rearrange("p (c f) -> p c f", c=C),
                            in_=_batch_ap(n, gb))
                    # y = x + 0.01*n (in-place into xt), one op over everything
                    nc.vector.scalar_tensor_tensor(
                        out=xt[:], in0=nt[:], scalar=NOISE_SCALE,
                        in1=xt[:], op0=Alu.mult, op1=Alu.add)

                    m = wk.tile([128, FD], _DT, tag="m")
                    mask = wk.tile([128, FD], mybir.dt.uint8, tag="mask")
                    ot = wk.tile([128, FD * C], _DT, tag="ot")
                    nc.vector.memset(m[:], 1e30)

                    def _kc(k, c):
                        if isinstance(k, int):
                            return slice(k * C + c, k * C + c + 1)
                        return ds(k * C + c, 1)

                    def _pal_row(k):
                        if isinstance(k, int):
                            s = pal_sb[:, k * C : (k + 1) * C]
                        else:
                            s = pal_sb[:, ds(k * C, C)]
                        return s.unsqueeze(1)

                    def kbody(k):
                        # q_c = (y_c - p_kc)^2 ; q reuses noise tile (nt)
                        for c in range(C):
                            nc.scalar.activation(
                                out=qview(nt, c), in_=qview(xt, c),
                                func=Act.Square,
                                bias=npal_sb[:, _kc(k, c)],
                                scale=1.0)
                        dv = qview(nt, 0)
                        nc.vector.tensor_tensor(
                            out=dv, in0=qview(nt, 0), in1=qview(nt, 1), op=Alu.add)
                        nc.vector.tensor_tensor(
                            out=dv, in0=dv, in1=qview(nt, 2), op=Alu.add)
                        mv = m[:].rearrange("p (b f) -> p b f", b=B_PER_UNIT)
                        mkv = mask[:].rearrange("p (b f) -> p b f", b=B_PER_UNIT)
                        nc.vector.tensor_tensor(
                            out=mkv, in0=dv, in1=mv, op=Alu.is_lt)
                        nc.vector.tensor_tensor(
                            out=mv, in0=mv, in1=dv, op=Alu.min)
                        # ot[(b f), c] = mask ? p_kc   (c-major views so no
                        # dim-merge: all three APs stay [p, 3, 4096])
                        nc.vector.copy_predicated(
                            out=ot[:].rearrange("p (f c) -> p c f", c=C),
                            mask=mask[:].unsqueeze(1).broadcast_to([128, C, FD]),
                            data=_pal_row(k).rearrange(
                                "p o c -> p c o").broadcast_to([128, C, FD]))

                    if kloop:
                        with tc.For_i(0, K) as kv:
                            kbody(kv)
                    else:
                        for k in range(K):
                            kbody(k)

                    # de-interleave ot -> planar (b,c,f) into nt
                    oti = ot[:].rearrange("p (b f c) -> p c b f", b=B_PER_UNIT, c=C)
                    for c in range(C):
                        nc.vector.tensor_copy(out=qview(nt, c), in_=oti[:, c])
                    for b in range(B_PER_UNIT):
                        gb = B_PER_UNIT * u + b
                        sl = slice(b * C * FB, (b + 1) * C * FB)
                        nc.sync.dma_start(
                            out=_batch_ap(o, gb),
                            in_=nt[:, sl].rearrange("p (c f) -> p c f", c=C))

            if loop and repeat > 1:
                with tc.For_i(0, repeat) as _i:
                    body(0)
            else:
                for rep in range(repeat):
                    body(rep)

    nc.compile()
    return nc


_NC_CACHE = {}


def _get_nc(repeat=1, loop=False):
    key = (repeat, loop)
    if key not in _NC_CACHE:
        _NC_CACHE[key] = _build(repeat, loop)
    return _NC_CACHE[key]


def kernel(x, noise, palette):
    x = np.ascontiguousarray(np.asarray(x, dtype=np.float32))
    noise = np.ascontiguousarray(np.asarray(noise, dtype=np.float32))
    palette = np.ascontiguousarray(np.asarray(palette, dtype=np.float32))
    nc = _get_nc()
    in_maps = [
        {
            "x": x[i * B_PER_CORE : (i + 1) * B_PER_CORE],
            "noise": noise[i * B_PER_CORE : (i + 1) * B_PER_CORE],
            "palette": palette,
        }
        for i in range(N_CORES)
    ]
    res = run_bass_kernel_spmd(nc, in_maps, list(range(N_CORES)))
    out = np.concatenate([res.results[i]["out"] for i in range(N_CORES)], axis=0)
    return out.astype(np.float32, copy=False)


# revision 3
# speedup vs baseline: 38.0126x; 1.2460x over previous
"""ColorQuantizer v3: per-instruction-overhead-optimized.

Empirical backend model: each instruction costs ~35-90us dispatch + ~6ns/free-el;
engines (DVE/ACT/DMA) run in parallel. So: minimize per-engine instruction
count, fd=4096 tiles (2 batches x 2048), optional For_i loop over palette
entries, 3MB DMAs, interleaved copy_predicated for selection.

Tile layouts per half (2 batches):
  xt/nt [128, 12288]: col = b*6144 + c*2048 + f   (planar per (b,c))
  ot    [128, 12288]: col = (b*2048+f)*3 + c      (pixel-interleaved)
"""
import sys

sys.path.insert(0, "/opt/trn_rl_repo")

import numpy as np

import concourse.bacc as bacc
import concourse.mybir as mybir
from concourse.tile import TileContext
from concourse.bass import ds
from concourse.bass_utils import run_bass_kernel_spmd

B, C, H, W = 32, 3, 512, 512
K = 16
N_CORES = 8
B_PER_CORE = B // N_CORES  # 4
NOISE_SCALE = 0.01

FD = 4096               # pixels per partition per half (2 batches x 2048)
B_PER_UNIT = 2
UNITS = B_PER_CORE // B_PER_UNIT  # 2
FB = 2048               # pixels per partition per batch

_DT = mybir.dt.float32
Alu = mybir.AluOpType
Act = mybir.ActivationFunctionType


def _batch_ap(t, b):
    """DRAM view [128, c, (a w)] of batch b: partition p <- h-rows [4p,4p+4)."""
    return t[b].rearrange("c (p a) w -> p c (a w)", p=128)


def _build(repeat=1, loop=False, kloop=False):
    nc = bacc.Bacc("TRN2", target_bir_lowering=False, debug=False,
                   num_devices=N_CORES)
    x = nc.dram_tensor("x", [B_PER_CORE, C, H, W], _DT, kind="ExternalInput").ap()
    n = nc.dram_tensor("noise", [B_PER_CORE, C, H, W], _DT, kind="ExternalInput").ap()
    pal = nc.dram_tensor("palette", [K, C], _DT, kind="ExternalInput").ap()
    o = nc.dram_tensor("out", [B_PER_CORE, C, H, W], _DT, kind="ExternalOutput").ap()

    with TileContext(nc) as tc:
        with (
            tc.tile_pool(name="const", bufs=1) as cpool,
            tc.tile_pool(name="io", bufs=1) as io,
            tc.tile_pool(name="work", bufs=1) as wk,
        ):
            # palette -> SBUF [128, 48] broadcast across partitions; col = k*3+c
            pal_sb = cpool.tile([128, K * C], _DT)
            nc.sync.dma_start(
                out=pal_sb[:],
                in_=pal.rearrange("(o k) c -> o (k c)", o=1).to_broadcast([128, K * C]),
            )
            npal_sb = cpool.tile([128, K * C], _DT)
            nc.vector.tensor_scalar(
                out=npal_sb[:], in0=pal_sb[:], scalar1=-1.0, scalar2=None,
                op0=Alu.mult)

            def qview(t, c):
                """channel-c view [128, b=2, 2048] of a (b,c,f)-layout tile."""
                return t[:].rearrange(
                    "p (b c f) -> p c b f", b=B_PER_UNIT, c=C)[:, c]

            def body(rep):
                for u in range(UNITS):
                    xt = io.tile([128, C * FD], _DT, tag="x", name="xt")
                    nt = io.tile([128, C * FD], _DT, tag="n", name="nt")
                    for b in range(B_PER_UNIT):
                        gb = B_PER_UNIT * u + b
                        sl = slice(b * C * FB, (b + 1) * C * FB)
                        nc.sync.dma_start(
                            out=xt[:, sl].rearrange("p (c f) -> p c f", c=C),
                            in_=_batch_ap(x, gb))
                        nc.sync.dma_start(
                            out=nt[:, sl].rearrange("p (c f) -> p c f", c=C),
                            in_=_batch_ap(n, gb))
                    # y = x + 0.01*n (in-place into xt), one op over everything
                    nc.vector.scalar_tensor_tensor(
                        out=xt[:], in0=nt[:], scalar=NOISE_SCALE,
                        in1=xt[:], op0=Alu.mult, op1=Alu.add)

                    m = wk.tile([128, FD], _DT, tag="m")
                    mask = wk.tile([128, FD], mybir.dt.uint8, tag="mask")
                    ot = wk.tile([128, FD * C], _DT, tag="ot")

                    def _kc(k, c):
                        if isinstance(k, int):
                            return slice(k * C + c, k * C + c + 1)
                        return ds(k * C + c, 1)

                    def _pal_row(k):
                        if isinstance(k, int):
                            s = pal_sb[:, k * C : (k + 1) * C]
                        else:
                            s = pal_sb[:, ds(k * C, C)]
                        return s.unsqueeze(1)

                    def kbody(k):
                        # q_c = (y_c - p_kc)^2 ; q reuses noise tile (nt)
                        for c in range(C):
                            nc.scalar.activation(
                                out=qview(nt, c), in_=qview(xt, c),
                                func=Act.Square,
                                bias=npal_sb[:, _kc(k, c)],
                                scale=1.0)
                        mv = m[:].rearrange("p (b f) -> p b f", b=B_PER_UNIT)
                        if k == 0:
                            # first candidate: m = d, ot = p_0 unconditionally
                            nc.vector.tensor_tensor(
                                out=mv, in0=qview(nt, 0), in1=qview(nt, 1),
                                op=Alu.add)
                            nc.vector.tensor_tensor(
                                out=mv, in0=mv, in1=qview(nt, 2), op=Alu.add)
                            nc.vector.tensor_copy(
                                out=ot[:].rearrange("p (f c) -> p c f", c=C),
                                in_=_pal_row(k).rearrange(
                                    "p o c -> p c o").broadcast_to([128, C, FD]))
                            return
                        dv = qview(nt, 0)
                        nc.vector.tensor_tensor(
                            out=dv, in0=qview(nt, 0), in1=qview(nt, 1), op=Alu.add)
                        nc.vector.tensor_tensor(
                            out=dv, in0=dv, in1=qview(nt, 2), op=Alu.add)
                        mkv = mask[:].rearrange("p (b f) -> p b f", b=B_PER_UNIT)
                        nc.vector.tensor_tensor(
                            out=mkv, in0=dv, in1=mv, op=Alu.is_lt)
                        nc.vector.tensor_tensor(
                            out=mv, in0=mv, in1=dv, op=Alu.min)
                        # ot[(b f), c] = mask ? p_kc   (c-major views so no
                        # dim-merge: all three APs stay [p, 3, 4096])
                        nc.vector.copy_predicated(
                            out=ot[:].rearrange("p (f c) -> p c f", c=C),
                            mask=mask[:].unsqueeze(1).broadcast_to([128, C, FD]),
                            data=_pal_row(k).rearrange(
                                "p o c -> p c o").broadcast_to([128, C, FD]))

                    if kloop:
                        with tc.For_i(0, K) as kv:
                            kbody(kv)
                    else:
                        for k in range(K):
                            kbody(k)

                    # de-interleave ot -> planar (b,c,f) into nt
                    oti = ot[:].rearrange("p (b f c) -> p c b f", b=B_PER_UNIT, c=C)
                    for c in range(C):
                        nc.scalar.activation(
                            out=qview(nt, c), in_=oti[:, c], func=Act.Copy)
                    for b in range(B_PER_UNIT):
                        gb = B_PER_UNIT * u + b
                        sl = slice(b * C * FB, (b + 1) * C * FB)
                        nc.sync.dma_start(
                            out=_batch_ap(o, gb),
                            in_=nt[:, sl].rearrange("p (c f) -> p c f", c=C))

            if loop and repeat > 1:
                with tc.For_i(0, repeat) as _i:
                    body(0)
            else:
                for rep in range(repeat):
                    body(rep)

    nc.compile()
    return nc


_NC_CACHE = {}


def _get_nc(repeat=1, loop=False):
    key = (repeat, loop)
    if key not in _NC_CACHE:
        _NC_CACHE[key] = _build(repeat, loop)
    return _NC_CACHE[key]


def kernel(x, noise, palette):
    x = np.ascontiguousarray(np.asarray(x, dtype=np.float32))
    noise = np.ascontiguousarray(np.asarray(noise, dtype=np.float32))
    palette = np.ascontiguousarray(np.asarray(palette, dtype=np.float32))
    nc = _get_nc()
    in_maps = [
        {
            "x": x[i * B_PER_CORE : (i + 1) * B_PER_CORE],
            "noise": noise[i * B_PER_CORE : (i + 1) * B_PER_CORE],
            "palette": palette,
        }
        for i in range(N_CORES)
    ]
    res = run_bass_kernel_spmd(nc, in_maps, list(range(N_CORES)))
    out = np.concatenate([res.results[i]["out"] for i in range(N_CORES)], axis=0)
    return out.astype(np.float32, copy=False)


# revision 5
# speedup vs baseline: 68.4312x; 1.8002x over previous
"""ColorQuantizer v6: double-buffered q tiles (ACT||DVE overlap) + packed ot.

vs v4: squares for entry k+1 run on ACT while DVE scans entry k (two q tile
sets, no WAR stall); the selected color is stored as one int32 per pixel
(3x10-bit quantized channels) so copy_predicated moves 1/3 the elements and
SBUF fits the extra q buffer. Output error ~5e-4 rel (tolerance 2e-2).

SBUF/partition: xt 48K + qA 48K + qB 48K + m 16K + mask 4K + ot 16K = 180K.
"""
import sys

sys.path.insert(0, "/opt/trn_rl_repo")

import numpy as np

import concourse.bacc as bacc
import concourse.mybir as mybir
from concourse.tile import TileContext
from concourse.bass_utils import run_bass_kernel_spmd

B, C, H, W = 32, 3, 512, 512
K = 16
N_CORES = 8
B_PER_CORE = B // N_CORES  # 4
NOISE_SCALE = 0.01

FD = 4096               # pixels per partition per half (2 batches x 2048)
B_PER_UNIT = 2
UNITS = B_PER_CORE // B_PER_UNIT  # 2
FB = 2048
QBITS = 10
QMAX = (1 << QBITS) - 1  # 1023

_DT = mybir.dt.float32
_IT = mybir.dt.int32
Alu = mybir.AluOpType
Act = mybir.ActivationFunctionType


def _batch_ap(t, b):
    """DRAM view [128, c, (a w)] of batch b: partition p <- h-rows [4p,4p+4)."""
    return t[b].rearrange("c (p a) w -> p c (a w)", p=128)


def _build(repeat=1, loop=False):
    nc = bacc.Bacc("TRN2", target_bir_lowering=False, debug=False,
                   num_devices=N_CORES)
    x = nc.dram_tensor("x", [B_PER_CORE, C, H, W], _DT, kind="ExternalInput").ap()
    n = nc.dram_tensor("noise", [B_PER_CORE, C, H, W], _DT, kind="ExternalInput").ap()
    pal = nc.dram_tensor("palette", [K, C], _DT, kind="ExternalInput").ap()
    o = nc.dram_tensor("out", [B_PER_CORE, C, H, W], _DT, kind="ExternalOutput").ap()

    with TileContext(nc) as tc:
        with (
            tc.tile_pool(name="const", bufs=1) as cpool,
            tc.tile_pool(name="io", bufs=1) as io,
            tc.tile_pool(name="work", bufs=1) as wk,
        ):
            # palette -> SBUF [128, 48] broadcast across partitions; col = k*3+c
            pal_sb = cpool.tile([128, K * C], _DT)
            nc.sync.dma_start(
                out=pal_sb[:],
                in_=pal.rearrange("(o k) c -> o (k c)", o=1).to_broadcast([128, K * C]),
            )
            npal_sb = cpool.tile([128, K * C], _DT)
            nc.vector.tensor_scalar(
                out=npal_sb[:], in0=pal_sb[:], scalar1=-1.0, scalar2=None,
                op0=Alu.mult)
            # packed quantized palette [128, 16] int32: (q0<<20)|(q1<<10)|q2
            qc = [cpool.tile([128, K], _IT, name=f"qc{c}") for c in range(C)]
            ppk = cpool.tile([128, K], _IT)
            for c in range(C):
                # q_c = round-ish(p_c * 1023); +0.499 guards truncating convert
                nc.vector.tensor_scalar(
                    out=qc[c][:],
                    in0=pal_sb[:].rearrange("p (k c) -> p c k", c=C)[:, c],
                    scalar1=float(QMAX), scalar2=0.499,
                    op0=Alu.mult, op1=Alu.add)
            nc.vector.tensor_scalar(
                out=ppk[:], in0=qc[0][:], scalar1=2 * QBITS, scalar2=None,
                op0=Alu.logical_shift_left)
            nc.vector.tensor_scalar(
                out=qc[1][:], in0=qc[1][:], scalar1=QBITS, scalar2=None,
                op0=Alu.logical_shift_left)
            nc.vector.tensor_tensor(
                out=ppk[:], in0=ppk[:], in1=qc[1][:], op=Alu.bitwise_or)
            nc.vector.tensor_tensor(
                out=ppk[:], in0=ppk[:], in1=qc[2][:], op=Alu.bitwise_or)

            def chview(t, c):
                """channel-c view [128, b=2, 2048] of a (b,c,f)-layout tile."""
                return t[:].rearrange(
                    "p (b c f) -> p c b f", b=B_PER_UNIT, c=C)[:, c]

            def body(rep):
                for u in range(UNITS):
                    xt = io.tile([128, C * FD], _DT, tag="x", name="xt")
                    qt = [io.tile([128, C * FD], _DT, tag=f"q{j}", name=f"qt{j}")
                          for j in range(2)]
                    for b in range(B_PER_UNIT):
                        gb = B_PER_UNIT * u + b
                        sl = slice(b * C * FB, (b + 1) * C * FB)
                        nc.sync.dma_start(
                            out=xt[:, sl].rearrange("p (c f) -> p c f", c=C),
                            in_=_batch_ap(x, gb))
                        nc.sync.dma_start(
                            out=qt[0][:, sl].rearrange("p (c f) -> p c f", c=C),
                            in_=_batch_ap(n, gb))
                    # y = x + 0.01*n (in-place into xt)
                    nc.vector.scalar_tensor_tensor(
                        out=xt[:], in0=qt[0][:], scalar=NOISE_SCALE,
                        in1=xt[:], op0=Alu.mult, op1=Alu.add)

                    m = wk.tile([128, FD], _DT, tag="m")
                    mask = wk.tile([128, FD], mybir.dt.uint8, tag="mask")
                    ot = wk.tile([128, FD], _IT, tag="ot")

                    for k in range(K):
                        q = qt[k % 2]
                        for c in range(C):
                            nc.scalar.activation(
                                out=chview(q, c), in_=chview(xt, c),
                                func=Act.Square,
                                bias=npal_sb[:, k * C + c : k * C + c + 1],
                                scale=1.0)
                        mv = m[:].rearrange("p (b f) -> p b f", b=B_PER_UNIT)
                        if k == 0:
                            nc.vector.tensor_tensor(
                                out=mv, in0=chview(q, 0), in1=chview(q, 1),
                                op=Alu.add)
                            nc.vector.tensor_tensor(
                                out=mv, in0=mv, in1=chview(q, 2), op=Alu.add)
                            nc.vector.tensor_copy(
                                out=ot[:],
                                in_=ppk[:, 0:1].to_broadcast([128, FD]))
                            continue
                        dv = chview(q, 0)
                        nc.vector.tensor_tensor(
                            out=dv, in0=chview(q, 0), in1=chview(q, 1), op=Alu.add)
                        nc.vector.tensor_tensor(
                            out=dv, in0=dv, in1=chview(q, 2), op=Alu.add)
                        mkv = mask[:].rearrange("p (b f) -> p b f", b=B_PER_UNIT)
                        nc.vector.tensor_tensor(
                            out=mkv, in0=dv, in1=mv, op=Alu.is_lt)
                        nc.vector.tensor_tensor(
                            out=mv, in0=mv, in1=dv, op=Alu.min)
                        nc.vector.copy_predicated(
                            out=ot[:], mask=mask[:],
                            data=ppk[:, k : k + 1].to_broadcast([128, FD]))

                    # unpack ot -> planar f32 channels into qt[1]: frees
                    # qt[0]/xt for the next half's DMAs during this tail
                    shifts = [2 * QBITS, QBITS, 0]
                    mi = m[:].bitcast(_IT)  # m dead; reuse as int scratch
                    for c in range(C):
                        nc.vector.tensor_scalar(
                            out=mi, in0=ot[:], scalar1=shifts[c], scalar2=QMAX,
                            op0=Alu.logical_shift_right, op1=Alu.bitwise_and)
                        nc.vector.tensor_scalar(
                            out=chview(qt[1], c),
                            in0=mi, scalar1=1.0 / QMAX, scalar2=None,
                            op0=Alu.mult)
                    for b in range(B_PER_UNIT):
                        gb = B_PER_UNIT * u + b
                        sl = slice(b * C * FB, (b + 1) * C * FB)
                        nc.sync.dma_start(
                            out=_batch_ap(o, gb),
                            in_=qt[1][:, sl].rearrange("p (c f) -> p c f", c=C))

            if loop and repeat > 1:
                with tc.For_i(0, repeat) as _i:
                    body(0)
            else:
                for rep in range(repeat):
                    body(rep)

    nc.compile()
    return nc


_NC_CACHE = {}


def _get_nc(repeat=1, loop=False):
    key = (repeat, loop)
    if key not in _NC_CACHE:
        _NC_CACHE[key] = _build(repeat, loop)
    return _NC_CACHE[key]


def kernel(x, noise, palette):
    x = np.ascontiguousarray(np.asarray(x, dtype=np.float32))
    noise = np.ascontiguousarray(np.asarray(noise, dtype=np.float32))
    palette = np.ascontiguousarray(np.asarray(palette, dtype=np.float32))
    nc = _get_nc()
    in_maps = [
        {
            "x": x[i * B_PER_CORE : (i + 1) * B_PER_CORE],
            "noise": noise[i * B_PER_CORE : (i + 1) * B_PER_CORE],
            "palette": palette,
        }
        for i in range(N_CORES)
    ]
    res = run_bass_kernel_spmd(nc, in_maps, list(range(N_CORES)))
    out = np.concatenate([res.results[i]["out"] for i in range(N_CORES)], axis=0)
    return out.astype(np.float32, copy=False)
